# revision 3
# baseline (speedup 1.0000x reference)
"""Trainium2 Bass kernel for nn_LinearReLU_19 (dense MLP + log_softmax).

Contract: kernel(**inputs) takes the FULL unsharded inputs (numpy, f32) and
returns the FULL [65536, 10] f32 output. Internally: data-parallel over 8
NeuronCores (batch sharded 8 x 8192), tiny weights replicated.

Math notes:
  - reference: h = x@w0.T (NO relu) ; h = relu(h@w1.T) ; ... so the first two
    linears fuse on the host into one [32, 784] matmul: relu(x @ (w1@w0).T).
  - activations are kept feature-major [feat, batch] on-chip so every matmul
    contracts over SBUF partitions; the last matmul uses the activation block
    as the stationary operand to emit batch-major [128, 10] logits directly.
  - log_softmax = logits - ln(sum(exp(logits))) computed per batch row with
    ScalarE Exp(accum_out=rowsum) + Ln + VectorE per-partition subtract.
    (No max-subtraction: |logits| <= O(10) so exp is safely in f32 range.)
  - matmuls run as float32r (full-rate fp32 mode on the PE at N>=256).
"""

import numpy as np

import concourse.bass as bass
import concourse.mybir as mybir
import concourse.tile as tile
import bass_rust
from concourse.bass_utils import run_bass_kernel_spmd

N_CORES = 8
B = 65536
BC = B // N_CORES            # 8192 rows per core
D_IN = 784
H1, H3, H5, D_OUT = 32, 22, 21, 10
K_CHUNK = 128                # contraction chunk (SBUF partitions)
N_CHUNKS = (D_IN + K_CHUNK - 1) // K_CHUNK   # 7 (6 full + 1 of 16)
NB = 2048                    # batch columns per DMA group
NSUB = 512                   # matmul moving free dim (fp32 max)
F32 = mybir.dt.float32
F32R = mybir.dt.float32r
ACTF = mybir.ActivationFunctionType


def _split_excess_waits(nc, max_waits=1):
    """walrus (CoreV3) accepts at most one sync-wait per instruction; move
    excess waits onto preceding same-engine no-ops (sequencer stalls on each,
    so barrier semantics are unchanged)."""
    for fn in nc.m.functions:
        for blk in fn.blocks:
            newlist = []
            changed = False
            for ins in blk.instructions:
                si = ins.sync_info
                if si is not None and len(si.on_wait) > max_waits:
                    waits = list(si.on_wait)
                    extra, keep = waits[:-max_waits], waits[-max_waits:]
                    for ci in range(0, len(extra), max_waits):
                        chunk = extra[ci:ci + max_waits]
                        nop = mybir.InstNoOp(
                            name=f"{ins.name}-waitsplit-{ci}", ins=[], outs=[])
                        nop.engine = ins.engine
                        nop.sync_info = bass_rust.SyncInfo(
                            on_wait=chunk, on_update=[])
                        newlist.append(nop)
                    ins.sync_info = bass_rust.SyncInfo(
                        on_wait=keep, on_update=list(si.on_update))
                    changed = True
                newlist.append(ins)
            if changed:
                blk.instructions[:] = newlist


def _kc(c):
    return min(K_CHUNK, D_IN - c * K_CHUNK)


def build_program():
    nc = bass.Bass("TRN2", target_bir_lowering=False, debug=False,
                   num_devices=N_CORES)

    xt_d = nc.dram_tensor("xt", [D_IN, BC], F32R, kind="ExternalInput").ap()
    wp_d = nc.dram_tensor("wpT", [D_IN, H1], F32R, kind="ExternalInput").ap()
    w3_d = nc.dram_tensor("w3T", [H1, H3], F32R, kind="ExternalInput").ap()
    w5_d = nc.dram_tensor("w5T", [H3, H5], F32R, kind="ExternalInput").ap()
    w7_d = nc.dram_tensor("w7T", [H5, D_OUT], F32R, kind="ExternalInput").ap()
    b3_d = nc.dram_tensor("b3", [H3, 1], F32, kind="ExternalInput").ap()
    b5_d = nc.dram_tensor("b5", [H5, 1], F32, kind="ExternalInput").ap()
    out_d = nc.dram_tensor("out", [BC, D_OUT], F32, kind="ExternalOutput").ap()

    r = lambda ap: ap

    with tile.TileContext(nc) as tc:
        with tc.tile_pool(name="consts", bufs=1) as cpool, \
             tc.tile_pool(name="data", bufs=3) as dpool, \
             tc.tile_pool(name="small", bufs=4) as spool, \
             tc.tile_pool(name="psacc", bufs=2, space="PSUM") as ppool, \
             tc.tile_pool(name="pslog", bufs=2, space="PSUM") as lpool:

            wp_sb = cpool.tile([K_CHUNK, N_CHUNKS * H1], F32R)
            for c in range(N_CHUNKS):
                kc = _kc(c)
                nc.sync.dma_start(
                    out=wp_sb[:kc, c * H1:(c + 1) * H1],
                    in_=wp_d[c * K_CHUNK:c * K_CHUNK + kc, :])
            w3_sb = cpool.tile([H1, H3], F32R)
            nc.sync.dma_start(out=w3_sb, in_=w3_d)
            w5_sb = cpool.tile([H3, H5], F32R)
            nc.sync.dma_start(out=w5_sb, in_=w5_d)
            w7_sb = cpool.tile([H5, D_OUT], F32R)
            nc.sync.dma_start(out=w7_sb, in_=w7_d)
            b3_sb = cpool.tile([H3, 1], F32)
            nc.sync.dma_start(out=b3_sb, in_=b3_d)
            b5_sb = cpool.tile([H5, 1], F32)
            nc.sync.dma_start(out=b5_sb, in_=b5_d)

            for g in range(BC // NB):
                b0 = g * NB
                xt_sb = dpool.tile([K_CHUNK, N_CHUNKS * NB], F32R, tag="xt")
                for c in range(N_CHUNKS):
                    kc = _kc(c)
                    nc.sync.dma_start(
                        out=xt_sb[:kc, c * NB:(c + 1) * NB],
                        in_=xt_d[c * K_CHUNK:c * K_CHUNK + kc, b0:b0 + NB])

                out_sb = spool.tile([128, (NB // 128) * D_OUT], F32, tag="out")

                for s in range(NB // NSUB):
                    n0 = s * NSUB
                    ps_h = ppool.tile([H1, NSUB], F32, tag="ps_h")
                    for c in range(N_CHUNKS):
                        kc = _kc(c)
                        nc.tensor.matmul(
                            ps_h,
                            lhsT=r(wp_sb[:kc, c * H1:(c + 1) * H1]),
                            rhs=r(xt_sb[:kc, c * NB + n0:c * NB + n0 + NSUB]),
                            start=(c == 0), stop=(c == N_CHUNKS - 1))
                    h1 = dpool.tile([H1, NSUB], F32R, tag="h1")
                    nc.scalar.activation(h1, ps_h, ACTF.Relu)

                    ps_3 = ppool.tile([H3, NSUB], F32, tag="ps_3")
                    nc.tensor.matmul(ps_3, lhsT=r(w3_sb), rhs=r(h1),
                                     start=True, stop=True)
                    h3 = dpool.tile([H3, NSUB], F32R, tag="h3")
                    nc.scalar.activation(h3, ps_3, ACTF.Relu, bias=b3_sb)

                    ps_5 = ppool.tile([H5, NSUB], F32, tag="ps_5")
                    nc.tensor.matmul(ps_5, lhsT=r(w5_sb), rhs=r(h3),
                                     start=True, stop=True)
                    h5 = dpool.tile([H5, NSUB], F32R, tag="h5")
                    nc.scalar.activation(h5, ps_5, ACTF.Relu, bias=b5_sb)

                    for j in range(NSUB // 128):
                        ps_t = lpool.tile([128, D_OUT], F32, tag="ps_t")
                        nc.tensor.matmul(
                            ps_t, lhsT=r(h5[:, j * 128:(j + 1) * 128]),
                            rhs=r(w7_sb), start=True, stop=True)
                        e_sb = spool.tile([128, D_OUT], F32, tag="e")
                        ssum = spool.tile([128, 1], F32, tag="ssum")
                        nc.scalar.activation(e_sb, ps_t, ACTF.Exp,
                                             accum_out=ssum)
                        ls = spool.tile([128, 1], F32, tag="ls")
                        nc.scalar.activation(ls, ssum, ACTF.Ln)
                        col = (s * (NSUB // 128) + j) * D_OUT
                        nc.vector.tensor_scalar_sub(
                            out=out_sb[:, col:col + D_OUT], in0=ps_t,
                            scalar1=ls)

                nc.sync.dma_start(
                    out=out_d[b0:b0 + NB].rearrange("(j p) f -> p j f", p=128),
                    in_=out_sb.rearrange("p (j f) -> p j f", f=D_OUT))

    _split_excess_waits(nc)
    return nc


_PROGRAM = None


def _get_program():
    global _PROGRAM
    if _PROGRAM is None:
        _PROGRAM = build_program()
    return _PROGRAM


def _prep_inputs(x, w0, w1, w3, b3, w5, b5, w7):
    x = np.ascontiguousarray(np.asarray(x, np.float32))
    wp = (np.asarray(w1, np.float64) @ np.asarray(w0, np.float64))
    wpT = np.ascontiguousarray(wp.T.astype(np.float32))          # [784, 32]
    w3T = np.ascontiguousarray(np.asarray(w3, np.float32).T)     # [32, 22]
    w5T = np.ascontiguousarray(np.asarray(w5, np.float32).T)     # [22, 21]
    w7T = np.ascontiguousarray(np.asarray(w7, np.float32).T)     # [21, 10]
    b3c = np.ascontiguousarray(np.asarray(b3, np.float32).reshape(H3, 1))
    b5c = np.ascontiguousarray(np.asarray(b5, np.float32).reshape(H5, 1))
    in_maps = []
    for c in range(N_CORES):
        xs = np.ascontiguousarray(x[c * BC:(c + 1) * BC].T)      # [784, 8192]
        in_maps.append({"xt": xs, "wpT": wpT, "w3T": w3T, "w5T": w5T,
                        "w7T": w7T, "b3": b3c, "b5": b5c})
    return in_maps


def run(in_maps, trace=False):
    nc = _get_program()
    res = run_bass_kernel_spmd(nc, in_maps, list(range(N_CORES)), trace=trace)
    out = np.concatenate([res.results[c]["out"] for c in range(N_CORES)],
                         axis=0)
    return out, res


def kernel(x, w0, w1, w3, b3, w5, b5, w7):
    in_maps = _prep_inputs(x, w0, w1, w3, b3, w5, b5, w7)
    out, _ = run(in_maps, trace=False)
    return out


# revision 4
# speedup vs baseline: 1.4742x; 1.4742x over previous
"""Trainium2 Bass kernel for nn_LinearReLU_19 (dense MLP + log_softmax).

Contract: kernel(**inputs) takes the FULL unsharded inputs (numpy, f32) and
returns the FULL [65536, 10] f32 output. Internally: data-parallel over 8
NeuronCores (batch sharded 8 x 8192), tiny weights replicated.

Design notes:
  - reference: h = x@w0.T (NO relu) ; h = relu(h@w1.T) ; ... so the first two
    linears fuse on the host into one [32, 784] matmul: relu(x @ (w1@w0).T).
  - x is transposed on the host so the contraction dim lands on SBUF
    partitions with fully contiguous DMA loads; activations stay
    feature-major [feat, batch] on-chip.
  - x is cast f32 -> bf16 in-flight by the SWDGE DMA (HBM still reads the
    full f32 input: the memory-bound workload is unchanged). Weights are
    pre-cast to bf16 on the host. bf16 matmuls run at 1 cycle/row on the PE
    (fp32/fp32r run at the throttled-clock / multi-pass rate).
  - the last matmul uses the activation block as the stationary operand to
    emit batch-major [128, 4*10] logits directly (no transpose needed).
  - log_softmax = logits - ln(sum(exp(logits))) per batch row:
    ScalarE Exp -> VectorE free-dim reduce -> ScalarE Ln -> VectorE
    broadcast subtract. (No max-subtraction: |logits| <= O(10), exp is
    safely inside f32 range.)
"""

import numpy as np
import ml_dtypes

import concourse.bass as bass
import concourse.mybir as mybir
import concourse.tile as tile
import bass_rust
from concourse.bass_utils import run_bass_kernel_spmd

N_CORES = 8
B = 65536
BC = B // N_CORES            # 8192 rows per core
D_IN = 784
H1, H3, H5, D_OUT = 32, 22, 21, 10
K_CHUNK = 128                # contraction chunk (SBUF partitions)
N_CHUNKS = (D_IN + K_CHUNK - 1) // K_CHUNK   # 7 (6 full + 1 of 16)
NB = 2048                    # batch columns per DMA group
NSUB = 512                   # matmul moving free dim
F32 = mybir.dt.float32
BF16 = mybir.dt.bfloat16
ACTF = mybir.ActivationFunctionType
BLK = NSUB // 128            # 4 logit blocks per subtile


def _split_excess_waits(nc, max_waits=1):
    """walrus (CoreV3) accepts at most one sync-wait per instruction; move
    excess waits onto preceding same-engine no-ops (sequencer stalls on each,
    so barrier semantics are unchanged)."""
    for fn in nc.m.functions:
        for blk in fn.blocks:
            newlist = []
            changed = False
            for ins in blk.instructions:
                si = ins.sync_info
                if si is not None and len(si.on_wait) > max_waits:
                    waits = list(si.on_wait)
                    extra, keep = waits[:-max_waits], waits[-max_waits:]
                    for ci in range(0, len(extra), max_waits):
                        chunk = extra[ci:ci + max_waits]
                        nop = mybir.InstNoOp(
                            name=f"{ins.name}-waitsplit-{ci}", ins=[], outs=[])
                        nop.engine = ins.engine
                        nop.sync_info = bass_rust.SyncInfo(
                            on_wait=chunk, on_update=[])
                        newlist.append(nop)
                    ins.sync_info = bass_rust.SyncInfo(
                        on_wait=keep, on_update=list(si.on_update))
                    changed = True
                newlist.append(ins)
            if changed:
                blk.instructions[:] = newlist


def _kc(c):
    return min(K_CHUNK, D_IN - c * K_CHUNK)


def build_program():
    nc = bass.Bass("TRN2", target_bir_lowering=False, debug=False,
                   num_devices=N_CORES)

    xt_d = nc.dram_tensor("xt", [D_IN, BC], F32, kind="ExternalInput").ap()
    wp_d = nc.dram_tensor("wpT", [D_IN, H1], BF16, kind="ExternalInput").ap()
    w3_d = nc.dram_tensor("w3T", [H1, H3], BF16, kind="ExternalInput").ap()
    w5_d = nc.dram_tensor("w5T", [H3, H5], BF16, kind="ExternalInput").ap()
    w7_d = nc.dram_tensor("w7T", [H5, D_OUT], BF16, kind="ExternalInput").ap()
    b3_d = nc.dram_tensor("b3", [H3, 1], F32, kind="ExternalInput").ap()
    b5_d = nc.dram_tensor("b5", [H5, 1], F32, kind="ExternalInput").ap()
    out_d = nc.dram_tensor("out", [BC, D_OUT], F32, kind="ExternalOutput").ap()

    with tile.TileContext(nc) as tc:
        with tc.tile_pool(name="consts", bufs=1) as cpool, \
             tc.tile_pool(name="data", bufs=3) as dpool, \
             tc.tile_pool(name="small", bufs=4) as spool, \
             tc.tile_pool(name="psacc", bufs=2, space="PSUM") as ppool, \
             tc.tile_pool(name="pslog", bufs=2, space="PSUM") as lpool:

            wp_sb = cpool.tile([K_CHUNK, N_CHUNKS * H1], BF16)
            for c in range(N_CHUNKS):
                kc = _kc(c)
                nc.sync.dma_start(
                    out=wp_sb[:kc, c * H1:(c + 1) * H1],
                    in_=wp_d[c * K_CHUNK:c * K_CHUNK + kc, :])
            w3_sb = cpool.tile([H1, H3], BF16)
            nc.sync.dma_start(out=w3_sb, in_=w3_d)
            w5_sb = cpool.tile([H3, H5], BF16)
            nc.sync.dma_start(out=w5_sb, in_=w5_d)
            w7_sb = cpool.tile([H5, D_OUT], BF16)
            nc.sync.dma_start(out=w7_sb, in_=w7_d)
            b3_sb = cpool.tile([H3, 1], F32)
            nc.sync.dma_start(out=b3_sb, in_=b3_d)
            b5_sb = cpool.tile([H5, 1], F32)
            nc.sync.dma_start(out=b5_sb, in_=b5_d)

            for g in range(BC // NB):
                b0 = g * NB
                xt_sb = dpool.tile([K_CHUNK, N_CHUNKS * NB], BF16, tag="xt")
                for c in range(N_CHUNKS):
                    kc = _kc(c)
                    # f32 -> bf16 cast happens inside the SWDGE DMA datapath
                    nc.gpsimd.dma_start(
                        out=xt_sb[:kc, c * NB:(c + 1) * NB],
                        in_=xt_d[c * K_CHUNK:c * K_CHUNK + kc, b0:b0 + NB])

                out_sb = spool.tile([128, (NB // 128) * D_OUT], F32, tag="out")

                for s in range(NB // NSUB):
                    n0 = s * NSUB
                    ps_h = ppool.tile([H1, NSUB], F32, tag="ps_h")
                    for c in range(N_CHUNKS):
                        kc = _kc(c)
                        nc.tensor.matmul(
                            ps_h,
                            lhsT=wp_sb[:kc, c * H1:(c + 1) * H1],
                            rhs=xt_sb[:kc, c * NB + n0:c * NB + n0 + NSUB],
                            start=(c == 0), stop=(c == N_CHUNKS - 1))
                    h1 = dpool.tile([H1, NSUB], BF16, tag="h1")
                    nc.vector.tensor_scalar_max(out=h1, in0=ps_h, scalar1=0.0)

                    ps_3 = ppool.tile([H3, NSUB], F32, tag="ps_3")
                    nc.tensor.matmul(ps_3, lhsT=w3_sb, rhs=h1,
                                     start=True, stop=True)
                    h3 = dpool.tile([H3, NSUB], BF16, tag="h3")
                    nc.scalar.activation(h3, ps_3, ACTF.Relu, bias=b3_sb)

                    ps_5 = ppool.tile([H5, NSUB], F32, tag="ps_5")
                    nc.tensor.matmul(ps_5, lhsT=w5_sb, rhs=h3,
                                     start=True, stop=True)
                    h5 = dpool.tile([H5, NSUB], BF16, tag="h5")
                    nc.scalar.activation(h5, ps_5, ACTF.Relu, bias=b5_sb)

                    # last layer: activations stationary -> batch-major logits
                    ps_t = lpool.tile([128, BLK * D_OUT], F32, tag="ps_t")
                    for j in range(BLK):
                        nc.tensor.matmul(
                            ps_t[:, j * D_OUT:(j + 1) * D_OUT],
                            lhsT=h5[:, j * 128:(j + 1) * 128],
                            rhs=w7_sb, start=True, stop=True)

                    e4 = spool.tile([128, BLK * D_OUT], F32, tag="e4")
                    nc.scalar.activation(e4, ps_t, ACTF.Exp)
                    s4 = spool.tile([128, BLK], F32, tag="s4")
                    nc.vector.tensor_reduce(
                        s4, e4.rearrange("p (j f) -> p j f", f=D_OUT),
                        axis=mybir.AxisListType.X, op=mybir.AluOpType.add)
                    ls4 = spool.tile([128, BLK], F32, tag="ls4")
                    nc.scalar.activation(ls4, s4, ACTF.Ln)
                    col = s * BLK * D_OUT
                    nc.vector.tensor_tensor(
                        out=out_sb[:, col:col + BLK * D_OUT].rearrange(
                            "p (j f) -> p j f", f=D_OUT),
                        in0=ps_t.rearrange("p (j f) -> p j f", f=D_OUT),
                        in1=ls4.broadcast_to([128, BLK, D_OUT]),
                        op=mybir.AluOpType.subtract)

                nc.sync.dma_start(
                    out=out_d[b0:b0 + NB].rearrange("(j p) f -> p j f", p=128),
                    in_=out_sb.rearrange("p (j f) -> p j f", f=D_OUT))

    _split_excess_waits(nc)
    return nc


_PROGRAM = None


def _get_program():
    global _PROGRAM
    if _PROGRAM is None:
        _PROGRAM = build_program()
    return _PROGRAM


def _prep_inputs(x, w0, w1, w3, b3, w5, b5, w7):
    x = np.ascontiguousarray(np.asarray(x, np.float32))
    bf = ml_dtypes.bfloat16
    wp = (np.asarray(w1, np.float64) @ np.asarray(w0, np.float64))
    wpT = np.ascontiguousarray(wp.T.astype(np.float32).astype(bf))  # [784,32]
    w3T = np.ascontiguousarray(np.asarray(w3, np.float32).T.astype(bf))
    w5T = np.ascontiguousarray(np.asarray(w5, np.float32).T.astype(bf))
    w7T = np.ascontiguousarray(np.asarray(w7, np.float32).T.astype(bf))
    b3c = np.ascontiguousarray(np.asarray(b3, np.float32).reshape(H3, 1))
    b5c = np.ascontiguousarray(np.asarray(b5, np.float32).reshape(H5, 1))
    in_maps = []
    for c in range(N_CORES):
        xs = np.ascontiguousarray(x[c * BC:(c + 1) * BC].T)      # [784, 8192]
        in_maps.append({"xt": xs, "wpT": wpT, "w3T": w3T, "w5T": w5T,
                        "w7T": w7T, "b3": b3c, "b5": b5c})
    return in_maps


def run(in_maps, trace=False):
    nc = _get_program()
    res = run_bass_kernel_spmd(nc, in_maps, list(range(N_CORES)), trace=trace)
    out = np.concatenate([res.results[c]["out"] for c in range(N_CORES)],
                         axis=0)
    return out, res


def kernel(x, w0, w1, w3, b3, w5, b5, w7):
    in_maps = _prep_inputs(x, w0, w1, w3, b3, w5, b5, w7)
    out, _ = run(in_maps, trace=False)
    return out
